# revision 5
# baseline (speedup 1.0000x reference)
"""Contrastive loss (video/audio) Trainium2 Bass kernel.

Full inputs: video [64,512,512] f32, audio [64,512,512] f32, mask [64,512] i32.
Data-parallel over batch: 8 cores x 8 batch elements. Each core computes its
partial loss sum on device; host adds the 8 scalars and divides by B.
"""

import numpy as np
from contextlib import ExitStack

import concourse.bass as bass
import concourse.tile as tile
from concourse import mybir
from concourse.bass_utils import run_bass_kernel_spmd

F32 = mybir.dt.float32
I32 = mybir.dt.int32
AF = mybir.ActivationFunctionType
OP = mybir.AluOpType
AX = mybir.AxisListType

B, T, D = 64, 512, 512
NCORES = 8
BL = B // NCORES          # 8 batch elements per core
P = 128                   # partitions
C = T // P                # 4 T-chunks per matrix
TEMP = 0.07

# engine split for the heavy per-chunk ops (tuned by profiling)
# r = row sum-of-squares; s = anchor-dot.  64 chunks each per core.
S_GPSIMD_FRAC = 0.0      # fraction of s-chunks on gpsimd instead of vector


def _flat(ap):
    """[p, a, b] -> [p, a*b]"""
    return ap.rearrange("p a b -> p (a b)")


def build_kernel(ctx: ExitStack, tc: tile.TileContext, video, audio, mask, out):
    nc = tc.nc

    persist = ctx.enter_context(tc.tile_pool(name="persist", bufs=1))
    data = ctx.enter_context(tc.tile_pool(name="data", bufs=3))
    anch = ctx.enter_context(tc.tile_pool(name="anch", bufs=3))
    scr = ctx.enter_context(tc.tile_pool(name="scr", bufs=2))
    psum = ctx.enter_context(tc.tile_pool(name="psum", bufs=2, space="PSUM"))
    dram = ctx.enter_context(tc.tile_pool(name="dram", bufs=1, space="DRAM"))

    # flat row views for the indirect gathers
    vrows = video.rearrange("b t d -> (b t) d")     # [4096, 512]
    arows = audio.rearrange("b t d -> (b t) d")

    # ---------------- argmax(mask) -> anchor row offsets -------------------
    mask_i = persist.tile([BL, T], I32, tag="mask_i")
    nc.sync.dma_start(mask_i[:], mask[:, :])
    mask_f = persist.tile([BL, T], F32, tag="mask_f")
    nc.vector.tensor_copy(mask_f[:], mask_i[:])
    iota_i = persist.tile([BL, T], I32, tag="iota_i")
    nc.gpsimd.iota(iota_i[:], pattern=[[1, T]], base=0, channel_multiplier=0)
    iota_f = persist.tile([BL, T], F32, tag="iota_f")
    nc.vector.tensor_copy(iota_f[:], iota_i[:])
    score = persist.tile([BL, T], F32, tag="score")
    # score = mask*1024 - t  (max picks first t with mask==1; exact in f32)
    nc.vector.scalar_tensor_tensor(
        out=score[:], in0=mask_f[:], scalar=1024.0, in1=iota_f[:],
        op0=OP.mult, op1=OP.subtract)
    maxs = persist.tile([BL, 1], F32, tag="maxs")
    nc.vector.reduce_max(maxs[:], score[:], axis=AX.X)
    idx_f = persist.tile([BL, 1], F32, tag="idx_f")
    nc.vector.tensor_scalar(
        out=idx_f[:], in0=maxs[:], scalar1=-1.0, scalar2=1024.0,
        op0=OP.mult, op1=OP.add)
    nc.vector.tensor_scalar_min(idx_f[:], idx_f[:], 511.0)
    brow_i = persist.tile([BL, 1], I32, tag="brow_i")
    nc.gpsimd.iota(brow_i[:], pattern=[[1, 1]], base=0, channel_multiplier=T)
    brow_f = persist.tile([BL, 1], F32, tag="brow_f")
    nc.vector.tensor_copy(brow_f[:], brow_i[:])
    row_f = persist.tile([BL, 1], F32, tag="row_f")
    nc.vector.tensor_add(row_f[:], idx_f[:], brow_f[:])
    row_i = persist.tile([BL, 1], I32, tag="row_i")
    nc.vector.tensor_copy(row_i[:], row_f[:])

    # ---------------- narrow anchor gathers: [8, 512] ----------------------
    anc_v = persist.tile([BL, D], F32, tag="anc_v")
    nc.gpsimd.indirect_dma_start(
        out=anc_v[:], out_offset=None, in_=vrows[:],
        in_offset=bass.IndirectOffsetOnAxis(ap=row_i[:, :1], axis=0))
    anc_a = persist.tile([BL, D], F32, tag="anc_a")
    nc.gpsimd.indirect_dma_start(
        out=anc_a[:], out_offset=None, in_=arows[:],
        in_offset=bass.IndirectOffsetOnAxis(ap=row_i[:, :1], axis=0))

    # anchor norms and inverse scales: inv = 1/(TEMP*||anchor||)
    nsc_v = persist.tile([BL, D], F32, tag="nsc_v")
    ra_v = persist.tile([BL, 1], F32, tag="ra_v")
    nc.vector.scalar_tensor_tensor(
        out=nsc_v[:], in0=anc_v[:], scalar=1.0, in1=anc_v[:],
        op0=OP.mult, op1=OP.mult, accum_out=ra_v[:])
    nsc_a = persist.tile([BL, D], F32, tag="nsc_a")
    ra_a = persist.tile([BL, 1], F32, tag="ra_a")
    nc.vector.scalar_tensor_tensor(
        out=nsc_a[:], in0=anc_a[:], scalar=1.0, in1=anc_a[:],
        op0=OP.mult, op1=OP.mult, accum_out=ra_a[:])
    sqa_v = persist.tile([BL, 1], F32, tag="sqa_v")
    nc.scalar.activation(sqa_v[:], ra_v[:], AF.Sqrt, scale=TEMP * TEMP)
    inv_v8 = persist.tile([BL, 1], F32, tag="inv_v8")
    nc.vector.reciprocal(inv_v8[:], sqa_v[:])
    sqa_a = persist.tile([BL, 1], F32, tag="sqa_a")
    nc.scalar.activation(sqa_a[:], ra_a[:], AF.Sqrt, scale=TEMP * TEMP)
    inv_a8 = persist.tile([BL, 1], F32, tag="inv_a8")
    nc.vector.reciprocal(inv_a8[:], sqa_a[:])

    # pos = (anc_v . anc_a) * inv_v * inv_a * TEMP
    pd_scr = persist.tile([BL, D], F32, tag="pd_scr")
    posd = persist.tile([BL, 1], F32, tag="posd")
    nc.vector.scalar_tensor_tensor(
        out=pd_scr[:], in0=anc_v[:], scalar=1.0, in1=anc_a[:],
        op0=OP.mult, op1=OP.mult, accum_out=posd[:])
    pos8 = persist.tile([BL, 1], F32, tag="pos8")
    nc.vector.tensor_tensor(pos8[:], posd[:], inv_v8[:], op=OP.mult)
    nc.vector.tensor_tensor(pos8[:], pos8[:], inv_a8[:], op=OP.mult)
    nc.vector.tensor_scalar_mul(pos8[:], pos8[:], TEMP)

    # ---------------- broadcast (row, inv_a, inv_v, pos) to all partitions -
    pack = persist.tile([BL, 4], F32, tag="pack")
    nc.vector.tensor_copy(pack[:, 0:1], row_f[:])
    nc.vector.tensor_copy(pack[:, 1:2], inv_a8[:])
    nc.vector.tensor_copy(pack[:, 2:3], inv_v8[:])
    nc.vector.tensor_copy(pack[:, 3:4], pos8[:])
    dscr = dram.tile([BL, 4], F32, tag="dscr")
    nc.sync.dma_start(dscr[:], pack[:])
    row0 = persist.tile([1, BL * 4], F32, tag="row0")
    nc.sync.dma_start(row0[:], dscr[:].rearrange("a b -> (a b)"))
    ones_row = persist.tile([1, P], F32, tag="ones_row")
    nc.vector.memset(ones_row[:], 1.0)
    bc_ps = psum.tile([P, BL * 4], F32, tag="bc_ps")
    nc.tensor.matmul(out=bc_ps[:], lhsT=ones_row[:], rhs=row0[:],
                     start=True, stop=True)
    bc = persist.tile([P, BL * 4], F32, tag="bc")
    nc.vector.tensor_copy(bc[:], bc_ps[:])
    bcv = bc[:].rearrange("p (a b) -> p a b", b=4)
    offs_f = persist.tile([P, BL], F32, tag="offs_f")
    nc.vector.tensor_copy(offs_f[:], _flat(bcv[:, :, 0:1]))
    offs_i = persist.tile([P, BL], I32, tag="offs_i")
    nc.vector.tensor_copy(offs_i[:], offs_f[:])

    # ---------------- main loop: r (ACT) and s (DVE/GPSIMD) ---------------
    rv_t = persist.tile([P, BL * C], F32, tag="rv_t")   # ||video_t||^2
    ra_t = persist.tile([P, BL * C], F32, tag="ra_t")   # ||audio_t||^2
    sa_t = persist.tile([P, BL * C], F32, tag="sa_t")   # video_t . anc_a * inv_a
    sv_t = persist.tile([P, BL * C], F32, tag="sv_t")   # audio_t . anc_v * inv_v

    vid_r = video.rearrange("b (c p) d -> b p c d", p=P)   # [8,128,4,512]
    aud_r = audio.rearrange("b (c p) d -> b p c d", p=P)

    for b in range(BL):
        vt = data.tile([P, C * D], F32, tag="vid")
        nc.sync.dma_start(vt[:].rearrange("p (c d) -> p c d", d=D), vid_r[b])
        at = data.tile([P, C * D], F32, tag="aud")
        nc.sync.dma_start(at[:].rearrange("p (c d) -> p c d", d=D), aud_r[b])
        # anchor rows replicated to all 128 partitions via repeated-offset gather
        abc = anch.tile([P, D], F32, tag="abc")
        nc.gpsimd.indirect_dma_start(
            out=abc[:], out_offset=None, in_=arows[:],
            in_offset=bass.IndirectOffsetOnAxis(ap=offs_i[:, b:b + 1], axis=0))
        vbc = anch.tile([P, D], F32, tag="vbc")
        nc.gpsimd.indirect_dma_start(
            out=vbc[:], out_offset=None, in_=vrows[:],
            in_offset=bass.IndirectOffsetOnAxis(ap=offs_i[:, b:b + 1], axis=0))
        ia_ap = _flat(bcv[:, b:b + 1, 1:2])   # [128,1] inv_a scale
        iv_ap = _flat(bcv[:, b:b + 1, 2:3])   # [128,1] inv_v scale
        for c in range(C):
            col = b * C + c
            vch = vt[:, c * D:(c + 1) * D]
            ach = at[:, c * D:(c + 1) * D]
            # row sum-of-squares on ACT (free accumulate)
            r1 = scr.tile([P, D], F32, tag="r1")
            nc.scalar.activation(r1[:], vch, AF.Square,
                                 accum_out=rv_t[:, col:col + 1])
            r2 = scr.tile([P, D], F32, tag="r2")
            nc.scalar.activation(r2[:], ach, AF.Square,
                                 accum_out=ra_t[:, col:col + 1])
            # anchor dots: (chunk * inv) * anchor_bcast, accumulated over D
            s1 = scr.tile([P, D], F32, tag="s1")
            nc.vector.scalar_tensor_tensor(
                out=s1[:], in0=vch, scalar=ia_ap, in1=abc[:],
                op0=OP.mult, op1=OP.mult, accum_out=sa_t[:, col:col + 1])
            s2 = scr.tile([P, D], F32, tag="s2")
            nc.vector.scalar_tensor_tensor(
                out=s2[:], in0=ach, scalar=iv_ap, in1=vbc[:],
                op0=OP.mult, op1=OP.mult, accum_out=sv_t[:, col:col + 1])

    # ---------------- post: scale, exp, reduce, combine --------------------
    ones = persist.tile([P, 1], F32, tag="ones")
    nc.vector.memset(ones[:], 1.0)

    srt_v = persist.tile([P, BL * C], F32, tag="srt_v")
    nc.scalar.activation(srt_v[:], rv_t[:], AF.Sqrt)
    irt_v = persist.tile([P, BL * C], F32, tag="irt_v")
    nc.vector.reciprocal(irt_v[:], srt_v[:])
    srt_a = persist.tile([P, BL * C], F32, tag="srt_a")
    nc.scalar.activation(srt_a[:], ra_t[:], AF.Sqrt)
    irt_a = persist.tile([P, BL * C], F32, tag="irt_a")
    nc.vector.reciprocal(irt_a[:], srt_a[:])

    ssc_a = persist.tile([P, BL * C], F32, tag="ssc_a")   # sims_a2v scaled
    nc.vector.tensor_tensor(ssc_a[:], sa_t[:], irt_v[:], op=OP.mult)
    ssc_v = persist.tile([P, BL * C], F32, tag="ssc_v")   # sims_v2a scaled
    nc.vector.tensor_tensor(ssc_v[:], sv_t[:], irt_a[:], op=OP.mult)

    exp_a = persist.tile([P, BL * C], F32, tag="exp_a")
    nc.scalar.activation(exp_a[:], ssc_a[:], AF.Exp)
    exp_v = persist.tile([P, BL * C], F32, tag="exp_v")
    nc.scalar.activation(exp_v[:], ssc_v[:], AF.Exp)

    pex_a = psum.tile([1, BL * C], F32, tag="pex_a")
    nc.tensor.matmul(out=pex_a[:], lhsT=ones[:], rhs=exp_a[:],
                     start=True, stop=True)
    pex_v = psum.tile([1, BL * C], F32, tag="pex_v")
    nc.tensor.matmul(out=pex_v[:], lhsT=ones[:], rhs=exp_v[:],
                     start=True, stop=True)

    se_a = persist.tile([1, BL], F32, tag="se_a")
    nc.vector.reduce_sum(
        se_a[:], pex_a[:].rearrange("p (a b) -> p a b", b=C), axis=AX.X)
    se_v = persist.tile([1, BL], F32, tag="se_v")
    nc.vector.reduce_sum(
        se_v[:], pex_v[:].rearrange("p (a b) -> p a b", b=C), axis=AX.X)

    pos_row = _flat(bcv[0:1, :, 3:4])            # [1,8]
    epos = persist.tile([1, BL], F32, tag="epos")
    nc.scalar.activation(epos[:], pos_row, AF.Exp)
    neg_a = persist.tile([1, BL], F32, tag="neg_a")
    nc.vector.tensor_tensor(neg_a[:], se_a[:], epos[:], op=OP.subtract)
    neg_v = persist.tile([1, BL], F32, tag="neg_v")
    nc.vector.tensor_tensor(neg_v[:], se_v[:], epos[:], op=OP.subtract)
    lg_a = persist.tile([1, BL], F32, tag="lg_a")
    nc.scalar.activation(lg_a[:], neg_a[:], AF.Ln)
    lg_v = persist.tile([1, BL], F32, tag="lg_v")
    nc.scalar.activation(lg_v[:], neg_v[:], AF.Ln)
    term = persist.tile([1, BL], F32, tag="term")
    nc.vector.tensor_tensor(term[:], lg_a[:], lg_v[:], op=OP.add)
    nc.vector.tensor_scalar_mul(term[:], term[:], 0.5)
    nc.vector.tensor_tensor(term[:], term[:], pos_row, op=OP.subtract)
    tot = persist.tile([1, 1], F32, tag="tot")
    nc.vector.reduce_sum(tot[:], term[:], axis=AX.X)
    nc.sync.dma_start(out[:, :], tot[:])


_CACHE = {}


def _get_nc():
    if "nc" not in _CACHE:
        nc = bass.Bass("TRN2", target_bir_lowering=False, debug=False,
                       num_devices=NCORES)
        video = nc.dram_tensor("video", [BL, T, D], F32,
                               kind="ExternalInput").ap()
        audio = nc.dram_tensor("audio", [BL, T, D], F32,
                               kind="ExternalInput").ap()
        mask = nc.dram_tensor("mask", [BL, T], I32, kind="ExternalInput").ap()
        out = nc.dram_tensor("out", [1, 1], F32, kind="ExternalOutput").ap()
        with tile.TileContext(nc) as tc:
            with ExitStack() as ctx:
                build_kernel(ctx, tc, video, audio, mask, out)
        from bir_legalize import legalize
        legalize(nc)
        _CACHE["nc"] = nc
    return _CACHE["nc"]


def kernel(video, audio, mask, _want_results=False):
    video = np.ascontiguousarray(np.asarray(video, dtype=np.float32))
    audio = np.ascontiguousarray(np.asarray(audio, dtype=np.float32))
    mask = np.ascontiguousarray(np.asarray(mask, dtype=np.int32))
    nc = _get_nc()
    in_maps = []
    for i in range(NCORES):
        sl = slice(i * BL, (i + 1) * BL)
        in_maps.append({"video": video[sl], "audio": audio[sl],
                        "mask": mask[sl]})
    res = run_bass_kernel_spmd(nc, in_maps, list(range(NCORES)))
    parts = [res.results[i]["out"][0, 0] for i in range(NCORES)]
    loss = np.float32(np.sum(np.asarray(parts, dtype=np.float64)) / B)
    outarr = np.asarray([loss], dtype=np.float32)
    if _want_results:
        return outarr, res
    return outarr


# revision 6
# speedup vs baseline: 1.3430x; 1.3430x over previous
"""Contrastive loss (video/audio) Trainium2 Bass kernel.

Full inputs: video [64,512,512] f32, audio [64,512,512] f32, mask [64,512] i32.
Data-parallel over batch: 8 cores x 8 batch elements. Each core computes its
partial loss sum on device; host adds the 8 scalars and divides by B.

Per-core pipeline:
  argmax(mask) by score trick -> anchor row ids -> narrow indirect gather
  [8,512] -> normalize (fold 1/TEMP) -> DRAM round-trip to a partition-0 row
  -> per-b PE outer-product broadcast (ones x anchor) into PSUM ->
  s = STT(video_chunk * anchor_bcast, accum) on DVE;
  r = ACT Square+accum;  exp on ACT;  partition-sum via PE ones-matvec;
  log/combine on partition 0; one scalar DMA'd out.
"""

import numpy as np
from contextlib import ExitStack

import concourse.bass as bass
import concourse.tile as tile
from concourse import mybir
from concourse.bass_utils import run_bass_kernel_spmd

F32 = mybir.dt.float32
I32 = mybir.dt.int32
AF = mybir.ActivationFunctionType
OP = mybir.AluOpType
AX = mybir.AxisListType

B, T, D = 64, 512, 512
NCORES = 8
BL = B // NCORES          # 8 batch elements per core
P = 128                   # partitions
C = T // P                # 4 T-chunks per matrix
TEMP = 0.07

# of the 64 r-chunks, how many go to DVE (STT) instead of ACT (square+accum)
R_ON_DVE = 0


def _flat(ap):
    return ap.rearrange("p a b -> p (a b)")


def build_kernel(ctx: ExitStack, tc: tile.TileContext, video, audio, mask, out):
    nc = tc.nc

    persist = ctx.enter_context(tc.tile_pool(name="persist", bufs=1))
    data = ctx.enter_context(tc.tile_pool(name="data", bufs=3))
    scr = ctx.enter_context(tc.tile_pool(name="scr", bufs=2))
    psum = ctx.enter_context(tc.tile_pool(name="psum", bufs=2, space="PSUM"))
    dram = ctx.enter_context(tc.tile_pool(name="dram", bufs=1, space="DRAM"))

    vrows = video.rearrange("b t d -> (b t) d")     # [4096, 512]
    arows = audio.rearrange("b t d -> (b t) d")

    # ---------------- argmax(mask) -> anchor row ids -----------------------
    mask_i = persist.tile([BL, T], I32, tag="mask_i")
    nc.sync.dma_start(mask_i[:], mask[:, :])
    mask_f = persist.tile([BL, T], F32, tag="mask_f")
    nc.vector.tensor_copy(mask_f[:], mask_i[:])
    iota_i = persist.tile([BL, T], I32, tag="iota_i")
    nc.gpsimd.iota(iota_i[:], pattern=[[1, T]], base=0, channel_multiplier=0)
    iota_f = persist.tile([BL, T], F32, tag="iota_f")
    nc.vector.tensor_copy(iota_f[:], iota_i[:])
    score = persist.tile([BL, T], F32, tag="score")
    # score = mask*1024 - t; max over t picks the first t with mask==1
    nc.vector.scalar_tensor_tensor(
        out=score[:], in0=mask_f[:], scalar=1024.0, in1=iota_f[:],
        op0=OP.mult, op1=OP.subtract)
    maxs = persist.tile([BL, 1], F32, tag="maxs")
    nc.vector.reduce_max(maxs[:], score[:], axis=AX.X)
    idx_f = persist.tile([BL, 1], F32, tag="idx_f")
    nc.vector.tensor_scalar(
        out=idx_f[:], in0=maxs[:], scalar1=-1.0, scalar2=1024.0,
        op0=OP.mult, op1=OP.add)
    nc.vector.tensor_scalar_min(idx_f[:], idx_f[:], 511.0)
    brow_i = persist.tile([BL, 1], I32, tag="brow_i")
    nc.gpsimd.iota(brow_i[:], pattern=[[1, 1]], base=0, channel_multiplier=T)
    brow_f = persist.tile([BL, 1], F32, tag="brow_f")
    nc.vector.tensor_copy(brow_f[:], brow_i[:])
    row_f = persist.tile([BL, 1], F32, tag="row_f")
    nc.vector.tensor_add(row_f[:], idx_f[:], brow_f[:])
    row_i = persist.tile([BL, 1], I32, tag="row_i")
    nc.vector.tensor_copy(row_i[:], row_f[:])

    # ---------------- narrow anchor gathers: [8, 512] ----------------------
    anc_v = persist.tile([BL, D], F32, tag="anc_v")
    nc.gpsimd.indirect_dma_start(
        out=anc_v[:], out_offset=None, in_=vrows[:],
        in_offset=bass.IndirectOffsetOnAxis(ap=row_i[:, :1], axis=0))
    anc_a = persist.tile([BL, D], F32, tag="anc_a")
    nc.gpsimd.indirect_dma_start(
        out=anc_a[:], out_offset=None, in_=arows[:],
        in_offset=bass.IndirectOffsetOnAxis(ap=row_i[:, :1], axis=0))

    # anchor norms; inv = 1/(TEMP*||anchor||) folded into the anchor rows
    nsc_v = persist.tile([BL, D], F32, tag="nsc_v")
    ra_v = persist.tile([BL, 1], F32, tag="ra_v")
    nc.vector.scalar_tensor_tensor(
        out=nsc_v[:], in0=anc_v[:], scalar=1.0, in1=anc_v[:],
        op0=OP.mult, op1=OP.mult, accum_out=ra_v[:])
    nsc_a = persist.tile([BL, D], F32, tag="nsc_a")
    ra_a = persist.tile([BL, 1], F32, tag="ra_a")
    nc.vector.scalar_tensor_tensor(
        out=nsc_a[:], in0=anc_a[:], scalar=1.0, in1=anc_a[:],
        op0=OP.mult, op1=OP.mult, accum_out=ra_a[:])
    sqa_v = persist.tile([BL, 1], F32, tag="sqa_v")
    nc.scalar.activation(sqa_v[:], ra_v[:], AF.Sqrt, scale=TEMP * TEMP)
    inv_v8 = persist.tile([BL, 1], F32, tag="inv_v8")
    nc.vector.reciprocal(inv_v8[:], sqa_v[:])
    sqa_a = persist.tile([BL, 1], F32, tag="sqa_a")
    nc.scalar.activation(sqa_a[:], ra_a[:], AF.Sqrt, scale=TEMP * TEMP)
    inv_a8 = persist.tile([BL, 1], F32, tag="inv_a8")
    nc.vector.reciprocal(inv_a8[:], sqa_a[:])

    anc_vs = persist.tile([BL, D], F32, tag="anc_vs")
    nc.vector.tensor_scalar_mul(anc_vs[:], anc_v[:], inv_v8[:, :1])
    anc_as = persist.tile([BL, D], F32, tag="anc_as")
    nc.vector.tensor_scalar_mul(anc_as[:], anc_a[:], inv_a8[:, :1])

    # pos = (anc_v . anc_a) * inv_v * inv_a * TEMP   (on partitions 0..7)
    pd_scr = persist.tile([BL, D], F32, tag="pd_scr")
    posd = persist.tile([BL, 1], F32, tag="posd")
    nc.vector.scalar_tensor_tensor(
        out=pd_scr[:], in0=anc_v[:], scalar=1.0, in1=anc_a[:],
        op0=OP.mult, op1=OP.mult, accum_out=posd[:])
    pos8 = persist.tile([BL, 1], F32, tag="pos8")
    nc.vector.tensor_tensor(pos8[:], posd[:], inv_v8[:], op=OP.mult)
    nc.vector.tensor_tensor(pos8[:], pos8[:], inv_a8[:], op=OP.mult)
    nc.vector.tensor_scalar_mul(pos8[:], pos8[:], TEMP)

    # ------- move scaled anchors + pos to partition-0 rows via DRAM --------
    d_av = dram.tile([BL, D], F32, tag="d_av")
    nc.sync.dma_start(d_av[:], anc_vs[:])
    d_aa = dram.tile([BL, D], F32, tag="d_aa")
    nc.sync.dma_start(d_aa[:], anc_as[:])
    d_pos = dram.tile([BL, 1], F32, tag="d_pos")
    nc.sync.dma_start(d_pos[:], pos8[:])
    row_av = persist.tile([1, BL * D], F32, tag="row_av")
    nc.sync.dma_start(row_av[:], d_av[:].rearrange("a b -> (a b)"))
    row_aa = persist.tile([1, BL * D], F32, tag="row_aa")
    nc.sync.dma_start(row_aa[:], d_aa[:].rearrange("a b -> (a b)"))
    pos_row = persist.tile([1, BL], F32, tag="pos_row")
    nc.sync.dma_start(pos_row[:], d_pos[:].rearrange("a b -> (a b)"))

    ones_row = persist.tile([1, P], F32, tag="ones_row")
    nc.vector.memset(ones_row[:], 1.0)
    ones_col = persist.tile([P, 1], F32, tag="ones_col")
    nc.vector.memset(ones_col[:], 1.0)

    # ---------------- main loop ---------------------------------------------
    rv_t = persist.tile([P, BL * C], F32, tag="rv_t")   # ||video_t||^2
    ra_t = persist.tile([P, BL * C], F32, tag="ra_t")   # ||audio_t||^2
    sa_t = persist.tile([P, BL * C], F32, tag="sa_t")   # video_t . a_anchor_s
    sv_t = persist.tile([P, BL * C], F32, tag="sv_t")   # audio_t . v_anchor_s

    vid_r = video.rearrange("b (c p) d -> b p c d", p=P)   # [8,128,4,512]
    aud_r = audio.rearrange("b (c p) d -> b p c d", p=P)

    for b in range(BL):
        vt = data.tile([P, C * D], F32, tag="vid")
        nc.sync.dma_start(vt[:].rearrange("p (c d) -> p c d", d=D), vid_r[b])
        at = data.tile([P, C * D], F32, tag="aud")
        nc.sync.dma_start(at[:].rearrange("p (c d) -> p c d", d=D), aud_r[b])
        # broadcast scaled anchors to all partitions: ones[128,1] @ row[1,512]
        abc = psum.tile([P, D], F32, tag="abc")
        nc.tensor.matmul(out=abc[:], lhsT=ones_row[:],
                         rhs=row_aa[:, b * D:(b + 1) * D],
                         start=True, stop=True)
        vbc = psum.tile([P, D], F32, tag="vbc")
        nc.tensor.matmul(out=vbc[:], lhsT=ones_row[:],
                         rhs=row_av[:, b * D:(b + 1) * D],
                         start=True, stop=True)
        for c in range(C):
            col = b * C + c
            vch = vt[:, c * D:(c + 1) * D]
            ach = at[:, c * D:(c + 1) * D]
            r1 = scr.tile([P, D], F32, tag="r1")
            nc.scalar.activation(r1[:], vch, AF.Square,
                                 accum_out=rv_t[:, col:col + 1])
            r2 = scr.tile([P, D], F32, tag="r2")
            nc.scalar.activation(r2[:], ach, AF.Square,
                                 accum_out=ra_t[:, col:col + 1])
            s1 = scr.tile([P, D], F32, tag="s1")
            nc.vector.scalar_tensor_tensor(
                out=s1[:], in0=vch, scalar=1.0, in1=abc[:],
                op0=OP.mult, op1=OP.mult, accum_out=sa_t[:, col:col + 1])
            s2 = scr.tile([P, D], F32, tag="s2")
            nc.vector.scalar_tensor_tensor(
                out=s2[:], in0=ach, scalar=1.0, in1=vbc[:],
                op0=OP.mult, op1=OP.mult, accum_out=sv_t[:, col:col + 1])

    # ---------------- post: scale, exp, reduce, combine --------------------
    srt_v = persist.tile([P, BL * C], F32, tag="srt_v")
    nc.scalar.activation(srt_v[:], rv_t[:], AF.Sqrt)
    irt_v = persist.tile([P, BL * C], F32, tag="irt_v")
    nc.vector.reciprocal(irt_v[:], srt_v[:])
    srt_a = persist.tile([P, BL * C], F32, tag="srt_a")
    nc.scalar.activation(srt_a[:], ra_t[:], AF.Sqrt)
    irt_a = persist.tile([P, BL * C], F32, tag="irt_a")
    nc.vector.reciprocal(irt_a[:], srt_a[:])

    ssc_a = persist.tile([P, BL * C], F32, tag="ssc_a")   # sims_a2v scaled
    nc.vector.tensor_tensor(ssc_a[:], sa_t[:], irt_v[:], op=OP.mult)
    ssc_v = persist.tile([P, BL * C], F32, tag="ssc_v")   # sims_v2a scaled
    nc.vector.tensor_tensor(ssc_v[:], sv_t[:], irt_a[:], op=OP.mult)

    exp_a = persist.tile([P, BL * C], F32, tag="exp_a")
    nc.scalar.activation(exp_a[:], ssc_a[:], AF.Exp)
    exp_v = persist.tile([P, BL * C], F32, tag="exp_v")
    nc.scalar.activation(exp_v[:], ssc_v[:], AF.Exp)

    pex_a = psum.tile([1, BL * C], F32, tag="pex_a")
    nc.tensor.matmul(out=pex_a[:], lhsT=ones_col[:], rhs=exp_a[:],
                     start=True, stop=True)
    pex_v = psum.tile([1, BL * C], F32, tag="pex_v")
    nc.tensor.matmul(out=pex_v[:], lhsT=ones_col[:], rhs=exp_v[:],
                     start=True, stop=True)

    se_a = persist.tile([1, BL], F32, tag="se_a")
    nc.vector.reduce_sum(
        se_a[:], pex_a[:].rearrange("p (a b) -> p a b", b=C), axis=AX.X)
    se_v = persist.tile([1, BL], F32, tag="se_v")
    nc.vector.reduce_sum(
        se_v[:], pex_v[:].rearrange("p (a b) -> p a b", b=C), axis=AX.X)

    epos = persist.tile([1, BL], F32, tag="epos")
    nc.scalar.activation(epos[:], pos_row[:], AF.Exp)
    neg_a = persist.tile([1, BL], F32, tag="neg_a")
    nc.vector.tensor_tensor(neg_a[:], se_a[:], epos[:], op=OP.subtract)
    neg_v = persist.tile([1, BL], F32, tag="neg_v")
    nc.vector.tensor_tensor(neg_v[:], se_v[:], epos[:], op=OP.subtract)
    lg_a = persist.tile([1, BL], F32, tag="lg_a")
    nc.scalar.activation(lg_a[:], neg_a[:], AF.Ln)
    lg_v = persist.tile([1, BL], F32, tag="lg_v")
    nc.scalar.activation(lg_v[:], neg_v[:], AF.Ln)
    term = persist.tile([1, BL], F32, tag="term")
    nc.vector.tensor_tensor(term[:], lg_a[:], lg_v[:], op=OP.add)
    nc.vector.tensor_scalar_mul(term[:], term[:], 0.5)
    nc.vector.tensor_tensor(term[:], term[:], pos_row[:], op=OP.subtract)
    tot = persist.tile([1, 1], F32, tag="tot")
    nc.vector.reduce_sum(tot[:], term[:], axis=AX.X)
    nc.sync.dma_start(out[:, :], tot[:])


_CACHE = {}


def _get_nc():
    if "nc" not in _CACHE:
        nc = bass.Bass("TRN2", target_bir_lowering=False, debug=False,
                       num_devices=NCORES)
        video = nc.dram_tensor("video", [BL, T, D], F32,
                               kind="ExternalInput").ap()
        audio = nc.dram_tensor("audio", [BL, T, D], F32,
                               kind="ExternalInput").ap()
        mask = nc.dram_tensor("mask", [BL, T], I32, kind="ExternalInput").ap()
        out = nc.dram_tensor("out", [1, 1], F32, kind="ExternalOutput").ap()
        with tile.TileContext(nc) as tc:
            with ExitStack() as ctx:
                build_kernel(ctx, tc, video, audio, mask, out)
        from bir_legalize import legalize
        legalize(nc)
        _CACHE["nc"] = nc
    return _CACHE["nc"]


def kernel(video, audio, mask, _want_results=False):
    video = np.ascontiguousarray(np.asarray(video, dtype=np.float32))
    audio = np.ascontiguousarray(np.asarray(audio, dtype=np.float32))
    mask = np.ascontiguousarray(np.asarray(mask, dtype=np.int32))
    nc = _get_nc()
    in_maps = []
    for i in range(NCORES):
        sl = slice(i * BL, (i + 1) * BL)
        in_maps.append({"video": video[sl], "audio": audio[sl],
                        "mask": mask[sl]})
    res = run_bass_kernel_spmd(nc, in_maps, list(range(NCORES)))
    parts = [res.results[i]["out"][0, 0] for i in range(NCORES)]
    loss = np.float32(np.sum(np.asarray(parts, dtype=np.float64)) / B)
    outarr = np.asarray([loss], dtype=np.float32)
    if _want_results:
        return outarr, res
    return outarr


# revision 9
# speedup vs baseline: 1.5942x; 1.1871x over previous
"""Contrastive loss (video/audio) Trainium2 Bass kernel.

Full inputs: video [64,512,512] f32, audio [64,512,512] f32, mask [64,512] i32.
Data-parallel over batch: 8 cores x 8 batch elements. Each core computes its
partial loss sum on device; host adds the 8 scalars and divides by B.

Per-core pipeline (v3):
  argmax(mask) via score trick -> anchor row ids -> narrow indirect gather
  [8,512] (raw anchors) -> per-b PE outer-product broadcast using an 8x8
  selector matrix (eye column broadcast as lhsT) -> PSUM [128,512] anchors.
  Main loop per b,c-chunk: r = ACT Square+accum; s_raw = DVE STT(mult,mult)
  with accum.  All normalization (sqrt/reciprocal), pos, exp, log and the
  final combine happen after the main loop; per-b anchor inverse norms are
  broadcast to 128 partitions with tiny PE outer-products and applied with a
  stride-0 free-dim AP.  One scalar partial sum is DMA'd out per core.
"""

import numpy as np
from contextlib import ExitStack

import concourse.bass as bass
import concourse.tile as tile
from concourse import mybir
from concourse.bass_utils import run_bass_kernel_spmd

F32 = mybir.dt.float32
I32 = mybir.dt.int32
AF = mybir.ActivationFunctionType
OP = mybir.AluOpType
AX = mybir.AxisListType

B, T, D = 64, 512, 512
NCORES = 8
BL = B // NCORES          # 8 batch elements per core
P = 128                   # partitions
C = T // P                # 4 T-chunks per matrix
TEMP = 0.07


def build_kernel(ctx: ExitStack, tc: tile.TileContext, video, audio, mask, out):
    nc = tc.nc

    persist = ctx.enter_context(tc.tile_pool(name="persist", bufs=1))
    data = ctx.enter_context(tc.tile_pool(name="data", bufs=3))
    scr = ctx.enter_context(tc.tile_pool(name="scr", bufs=2))
    psum = ctx.enter_context(tc.tile_pool(name="psum", bufs=2, space="PSUM"))
    psum1 = ctx.enter_context(tc.tile_pool(name="psum1", bufs=1, space="PSUM"))
    dram = ctx.enter_context(tc.tile_pool(name="dram", bufs=1, space="DRAM"))

    vrows = video.rearrange("b t d -> (b t) d")     # [4096, 512]
    arows = audio.rearrange("b t d -> (b t) d")

    # ---------------- argmax(mask) -> anchor row ids -----------------------
    mask_i = persist.tile([BL, T], I32, tag="mask_i")
    nc.sync.dma_start(mask_i[:], mask[:, :])
    mask_f = persist.tile([BL, T], F32, tag="mask_f")
    nc.vector.tensor_copy(mask_f[:], mask_i[:])
    iota_i = persist.tile([BL, T], I32, tag="iota_i")
    nc.gpsimd.iota(iota_i[:], pattern=[[1, T]], base=0, channel_multiplier=0)
    iota_f = persist.tile([BL, T], F32, tag="iota_f")
    nc.vector.tensor_copy(iota_f[:], iota_i[:])
    score = persist.tile([BL, T], F32, tag="score")
    nc.vector.scalar_tensor_tensor(
        out=score[:], in0=mask_f[:], scalar=1024.0, in1=iota_f[:],
        op0=OP.mult, op1=OP.subtract)
    maxs = persist.tile([BL, 1], F32, tag="maxs")
    nc.vector.reduce_max(maxs[:], score[:], axis=AX.X)
    idx_f = persist.tile([BL, 1], F32, tag="idx_f")
    nc.vector.tensor_scalar(
        out=idx_f[:], in0=maxs[:], scalar1=-1.0, scalar2=1024.0,
        op0=OP.mult, op1=OP.add)
    nc.vector.tensor_scalar_min(idx_f[:], idx_f[:], 511.0)
    brow_i = persist.tile([BL, 1], I32, tag="brow_i")
    nc.gpsimd.iota(brow_i[:], pattern=[[1, 1]], base=0, channel_multiplier=T)
    brow_f = persist.tile([BL, 1], F32, tag="brow_f")
    nc.vector.tensor_copy(brow_f[:], brow_i[:])
    row_f = persist.tile([BL, 1], F32, tag="row_f")
    nc.vector.tensor_add(row_f[:], idx_f[:], brow_f[:])
    row_i = persist.tile([BL, 1], I32, tag="row_i")
    nc.vector.tensor_copy(row_i[:], row_f[:])

    # 8x8 selector: eye8[k,m] = (k == m), as f32
    eyei = persist.tile([BL, BL], I32, tag="eyei")
    nc.gpsimd.iota(eyei[:], pattern=[[1, BL]], base=0, channel_multiplier=-1)
    eyez = persist.tile([BL, BL], I32, tag="eyez")
    nc.vector.tensor_scalar(out=eyez[:], in0=eyei[:], scalar1=0,
                            scalar2=None, op0=OP.is_equal)
    eyef = persist.tile([BL, BL], F32, tag="eyef")
    nc.vector.tensor_copy(eyef[:], eyez[:])

    # ---------------- narrow anchor gathers: [8, 512] (raw) ----------------
    anc_v = persist.tile([BL, D], F32, tag="anc_v")
    nc.gpsimd.indirect_dma_start(
        out=anc_v[:], out_offset=None, in_=vrows[:],
        in_offset=bass.IndirectOffsetOnAxis(ap=row_i[:, :1], axis=0))
    anc_a = persist.tile([BL, D], F32, tag="anc_a")
    nc.gpsimd.indirect_dma_start(
        out=anc_a[:], out_offset=None, in_=arows[:],
        in_offset=bass.IndirectOffsetOnAxis(ap=row_i[:, :1], axis=0))

    # ---------------- main loop ---------------------------------------------
    rv_t = persist.tile([P, BL * C], F32, tag="rv_t")   # ||video_t||^2
    ra_t = persist.tile([P, BL * C], F32, tag="ra_t")   # ||audio_t||^2
    sa_t = persist.tile([P, BL * C], F32, tag="sa_t")   # video_t . anc_a(raw)
    sv_t = persist.tile([P, BL * C], F32, tag="sv_t")   # audio_t . anc_v(raw)

    # contiguous-row tiling: t = 4*p + c -> one 8KB descriptor per partition
    vid_r = video.rearrange("b (p c) d -> b p c d", p=P)   # [8,128,4,512]
    aud_r = audio.rearrange("b (p c) d -> b p c d", p=P)

    for b in range(BL):
        vt = data.tile([P, C * D], F32, tag="vid")
        nc.sync.dma_start(vt[:].rearrange("p (c d) -> p c d", d=D), vid_r[b])
        at = data.tile([P, C * D], F32, tag="aud")
        nc.sync.dma_start(at[:].rearrange("p (c d) -> p c d", d=D), aud_r[b])
        # anchor_b broadcast to all partitions: sel_b[8,128].T @ anchors[8,512]
        sel = eyef[:, b:b + 1].to_broadcast([BL, P])
        abc = psum.tile([P, D], F32, tag="abc")
        nc.tensor.matmul(out=abc[:], lhsT=sel, rhs=anc_a[:],
                         start=True, stop=True)
        vbc = psum.tile([P, D], F32, tag="vbc")
        nc.tensor.matmul(out=vbc[:], lhsT=sel, rhs=anc_v[:],
                         start=True, stop=True)
        for c in range(C):
            col = b * C + c
            vch = vt[:, c * D:(c + 1) * D]
            ach = at[:, c * D:(c + 1) * D]
            r1 = scr.tile([P, D], F32, tag="r1")
            nc.scalar.activation(r1[:], vch, AF.Square,
                                 accum_out=rv_t[:, col:col + 1])
            r2 = scr.tile([P, D], F32, tag="r2")
            nc.scalar.activation(r2[:], ach, AF.Square,
                                 accum_out=ra_t[:, col:col + 1])
            s1 = scr.tile([P, D], F32, tag="s1")
            nc.vector.scalar_tensor_tensor(
                out=s1[:], in0=vch, scalar=1.0, in1=abc[:],
                op0=OP.mult, op1=OP.mult, accum_out=sa_t[:, col:col + 1])
            s2 = scr.tile([P, D], F32, tag="s2")
            nc.vector.scalar_tensor_tensor(
                out=s2[:], in0=ach, scalar=1.0, in1=vbc[:],
                op0=OP.mult, op1=OP.mult, accum_out=sv_t[:, col:col + 1])

    # ------- anchor norms, pos (computed late, off the critical path) ------
    nsc_v = persist.tile([BL, D], F32, tag="nsc_v")
    ran_v = persist.tile([BL, 1], F32, tag="ran_v")
    nc.vector.scalar_tensor_tensor(
        out=nsc_v[:], in0=anc_v[:], scalar=1.0, in1=anc_v[:],
        op0=OP.mult, op1=OP.mult, accum_out=ran_v[:])
    nsc_a = persist.tile([BL, D], F32, tag="nsc_a")
    ran_a = persist.tile([BL, 1], F32, tag="ran_a")
    nc.vector.scalar_tensor_tensor(
        out=nsc_a[:], in0=anc_a[:], scalar=1.0, in1=anc_a[:],
        op0=OP.mult, op1=OP.mult, accum_out=ran_a[:])
    pd_scr = persist.tile([BL, D], F32, tag="pd_scr")
    posd = persist.tile([BL, 1], F32, tag="posd")
    nc.vector.scalar_tensor_tensor(
        out=pd_scr[:], in0=anc_v[:], scalar=1.0, in1=anc_a[:],
        op0=OP.mult, op1=OP.mult, accum_out=posd[:])
    # round-trip (ran_a, ran_v, posd) to a partition-0 row [1, 24]
    pk = persist.tile([BL, 3], F32, tag="pk")
    nc.vector.tensor_copy(pk[:, 0:1], ran_a[:])
    nc.vector.tensor_copy(pk[:, 1:2], ran_v[:])
    nc.vector.tensor_copy(pk[:, 2:3], posd[:])
    d_pk = dram.tile([BL, 3], F32, tag="d_pk")
    nc.sync.dma_start(d_pk[:], pk[:])
    pkr = persist.tile([1, BL * 3], F32, tag="pkr")
    nc.sync.dma_start(pkr[:], d_pk[:].rearrange("a b -> (a b)"))
    pkr3 = pkr[:].rearrange("p (a b) -> p a b", b=3)
    raa_row = persist.tile([1, BL], F32, tag="raa_row")
    nc.vector.tensor_copy(raa_row[:], pkr3[:, :, 0:1].rearrange("p a b -> p (a b)"))
    rav_row = persist.tile([1, BL], F32, tag="rav_row")
    nc.vector.tensor_copy(rav_row[:], pkr3[:, :, 1:2].rearrange("p a b -> p (a b)"))
    pod_row = persist.tile([1, BL], F32, tag="pod_row")
    nc.vector.tensor_copy(pod_row[:], pkr3[:, :, 2:3].rearrange("p a b -> p (a b)"))
    # inv rows: 1/(TEMP*sqrt(r))
    sq_ra = persist.tile([1, BL], F32, tag="sq_ra")
    nc.scalar.activation(sq_ra[:], raa_row[:], AF.Sqrt, scale=TEMP * TEMP)
    inva_row = persist.tile([1, BL], F32, tag="inva_row")
    nc.vector.reciprocal(inva_row[:], sq_ra[:])
    sq_rv = persist.tile([1, BL], F32, tag="sq_rv")
    nc.scalar.activation(sq_rv[:], rav_row[:], AF.Sqrt, scale=TEMP * TEMP)
    invv_row = persist.tile([1, BL], F32, tag="invv_row")
    nc.vector.reciprocal(invv_row[:], sq_rv[:])
    # pos = posd * inv_a * inv_v * TEMP   on [1,8]
    pos_row = persist.tile([1, BL], F32, tag="pos_row")
    nc.vector.tensor_tensor(pos_row[:], pod_row[:], inva_row[:], op=OP.mult)
    nc.vector.tensor_tensor(pos_row[:], pos_row[:], invv_row[:], op=OP.mult)
    nc.vector.tensor_scalar_mul(pos_row[:], pos_row[:], TEMP)
    # broadcast inv rows to [128, 8] via PE outer
    ones_row = persist.tile([1, P], F32, tag="ones_row")
    nc.vector.memset(ones_row[:], 1.0)
    ones_col = persist.tile([P, 1], F32, tag="ones_col")
    nc.vector.memset(ones_col[:], 1.0)
    inv_bc = psum1.tile([P, 2 * BL], F32, tag="inv_bc")
    nc.tensor.matmul(out=inv_bc[:, 0:BL], lhsT=ones_row[:], rhs=inva_row[:],
                     start=True, stop=True)
    nc.tensor.matmul(out=inv_bc[:, BL:2 * BL], lhsT=ones_row[:],
                     rhs=invv_row[:], start=True, stop=True)
    inva_bc = inv_bc[:, 0:BL]
    invv_bc = inv_bc[:, BL:2 * BL]

    # ---------------- post: scale, exp, reduce, combine --------------------
    srt_v = persist.tile([P, BL * C], F32, tag="srt_v")
    nc.scalar.activation(srt_v[:], rv_t[:], AF.Sqrt)
    irt_v = persist.tile([P, BL * C], F32, tag="irt_v")
    nc.vector.reciprocal(irt_v[:], srt_v[:])
    srt_a = persist.tile([P, BL * C], F32, tag="srt_a")
    nc.scalar.activation(srt_a[:], ra_t[:], AF.Sqrt)
    irt_a = persist.tile([P, BL * C], F32, tag="irt_a")
    nc.vector.reciprocal(irt_a[:], srt_a[:])

    # combined scale: irt * anchor_inv(b)  (stride-0 broadcast over c)
    cmb_a = persist.tile([P, BL, C], F32, tag="cmb_a")
    nc.vector.tensor_tensor(
        cmb_a[:], irt_v[:].rearrange("p (a b) -> p a b", b=C),
        inva_bc.to_broadcast([P, BL, C]),
        op=OP.mult)
    cmb_v = persist.tile([P, BL, C], F32, tag="cmb_v")
    nc.vector.tensor_tensor(
        cmb_v[:], irt_a[:].rearrange("p (a b) -> p a b", b=C),
        invv_bc.to_broadcast([P, BL, C]),
        op=OP.mult)

    ssc_a = persist.tile([P, BL * C], F32, tag="ssc_a")
    nc.vector.tensor_tensor(ssc_a[:], sa_t[:],
                            cmb_a[:].rearrange("p a b -> p (a b)"), op=OP.mult)
    ssc_v = persist.tile([P, BL * C], F32, tag="ssc_v")
    nc.vector.tensor_tensor(ssc_v[:], sv_t[:],
                            cmb_v[:].rearrange("p a b -> p (a b)"), op=OP.mult)

    exp_a = persist.tile([P, BL * C], F32, tag="exp_a")
    nc.scalar.activation(exp_a[:], ssc_a[:], AF.Exp)
    exp_v = persist.tile([P, BL * C], F32, tag="exp_v")
    nc.scalar.activation(exp_v[:], ssc_v[:], AF.Exp)

    pex = psum1.tile([1, 2 * BL * C], F32, tag="pex")
    nc.tensor.matmul(out=pex[:, 0:BL * C], lhsT=ones_col[:], rhs=exp_a[:],
                     start=True, stop=True)
    nc.tensor.matmul(out=pex[:, BL * C:2 * BL * C], lhsT=ones_col[:],
                     rhs=exp_v[:], start=True, stop=True)
    pex_a = pex[:, 0:BL * C]
    pex_v = pex[:, BL * C:2 * BL * C]

    se_a = persist.tile([1, BL], F32, tag="se_a")
    nc.vector.reduce_sum(
        se_a[:], pex_a.rearrange("p (a b) -> p a b", b=C), axis=AX.X)
    se_v = persist.tile([1, BL], F32, tag="se_v")
    nc.vector.reduce_sum(
        se_v[:], pex_v.rearrange("p (a b) -> p a b", b=C), axis=AX.X)

    epos = persist.tile([1, BL], F32, tag="epos")
    nc.scalar.activation(epos[:], pos_row[:], AF.Exp)
    neg_a = persist.tile([1, BL], F32, tag="neg_a")
    nc.vector.tensor_tensor(neg_a[:], se_a[:], epos[:], op=OP.subtract)
    neg_v = persist.tile([1, BL], F32, tag="neg_v")
    nc.vector.tensor_tensor(neg_v[:], se_v[:], epos[:], op=OP.subtract)
    lg_a = persist.tile([1, BL], F32, tag="lg_a")
    nc.scalar.activation(lg_a[:], neg_a[:], AF.Ln)
    lg_v = persist.tile([1, BL], F32, tag="lg_v")
    nc.scalar.activation(lg_v[:], neg_v[:], AF.Ln)
    term = persist.tile([1, BL], F32, tag="term")
    nc.vector.tensor_tensor(term[:], lg_a[:], lg_v[:], op=OP.add)
    nc.vector.tensor_scalar_mul(term[:], term[:], 0.5)
    nc.vector.tensor_tensor(term[:], term[:], pos_row[:], op=OP.subtract)
    tot = persist.tile([1, 1], F32, tag="tot")
    nc.vector.reduce_sum(tot[:], term[:], axis=AX.X)
    nc.sync.dma_start(out[:, :], tot[:])


_CACHE = {}


def _get_nc():
    if "nc" not in _CACHE:
        nc = bass.Bass("TRN2", target_bir_lowering=False, debug=False,
                       num_devices=NCORES)
        video = nc.dram_tensor("video", [BL, T, D], F32,
                               kind="ExternalInput").ap()
        audio = nc.dram_tensor("audio", [BL, T, D], F32,
                               kind="ExternalInput").ap()
        mask = nc.dram_tensor("mask", [BL, T], I32, kind="ExternalInput").ap()
        out = nc.dram_tensor("out", [1, 1], F32, kind="ExternalOutput").ap()
        with tile.TileContext(nc) as tc:
            with ExitStack() as ctx:
                build_kernel(ctx, tc, video, audio, mask, out)
        from bir_legalize import legalize
        legalize(nc)
        _CACHE["nc"] = nc
    return _CACHE["nc"]


def kernel(video, audio, mask, _want_results=False):
    video = np.ascontiguousarray(np.asarray(video, dtype=np.float32))
    audio = np.ascontiguousarray(np.asarray(audio, dtype=np.float32))
    mask = np.ascontiguousarray(np.asarray(mask, dtype=np.int32))
    nc = _get_nc()
    in_maps = []
    for i in range(NCORES):
        sl = slice(i * BL, (i + 1) * BL)
        in_maps.append({"video": video[sl], "audio": audio[sl],
                        "mask": mask[sl]})
    res = run_bass_kernel_spmd(nc, in_maps, list(range(NCORES)))
    parts = [res.results[i]["out"][0, 0] for i in range(NCORES)]
    loss = np.float32(np.sum(np.asarray(parts, dtype=np.float64)) / B)
    outarr = np.asarray([loss], dtype=np.float32)
    if _want_results:
        return outarr, res
    return outarr
